# revision 1
# baseline (speedup 1.0000x reference)
"""LoRA linear on 8 Trainium2 NeuronCores.

out = x @ (W + A @ B)^T + bias
  x: [4, 4096, 4096] f32, W: [4096, 4096], bias: [4096], A: [4096, 16], B: [16, 4096]

Strategy (column-parallel / d_out-sharded, per the tensor-parallel pattern):
  - Host: Weff = W + A@B (0.1% of total FLOPs), pre-transpose x and Weff so the
    contraction dim lands on SBUF partitions with no on-chip transposes, round
    both to FP32R (fp32 with 11-bit mantissa; low 12 bits zero) so the PE runs
    matmuls at 1 cycle/row instead of fp32's 4.
  - Each core c: out[:, c*512:(c+1)*512] = x @ WeffT[:, c*512:(c+1)*512] + bias_c.
    WeffT shard (8 MB) stays SBUF-resident; xT streams in m-blocks; 32 k-tile
    matmuls accumulate in PSUM; bias add fused into the PSUM->SBUF evacuation.
"""
import numpy as np

import concourse.bacc as bacc
import concourse.mybir as mybir
import concourse.tile as tile
from concourse.bass_utils import run_bass_kernel_spmd

BATCH, SEQ, D = 4, 4096, 4096
M = BATCH * SEQ          # 16384 rows
K = D                    # contraction
N_CORES = 8
OS = D // N_CORES        # 512 output cols per core
KT = K // 128            # 32 k-tiles
MB = 256                 # m-block rows per x stream tile

_f32 = mybir.dt.float32
_f32r = mybir.dt.float32r

_COMPILED = None


def _build():
    nc = bacc.Bacc("TRN2", target_bir_lowering=False, debug=False,
                   num_devices=N_CORES)
    xT = nc.dram_tensor("xT", [K, M], _f32r, kind="ExternalInput").ap()
    wT = nc.dram_tensor("wT", [K, OS], _f32r, kind="ExternalInput").ap()
    bias = nc.dram_tensor("bias", [128, OS], _f32, kind="ExternalInput").ap()
    out = nc.dram_tensor("out", [M, OS], _f32, kind="ExternalOutput").ap()

    with tile.TileContext(nc) as tc:
        with tc.tile_pool(name="w", bufs=1) as wp, \
             tc.tile_pool(name="xb", bufs=3) as xp, \
             tc.tile_pool(name="ob", bufs=4) as op_, \
             tc.tile_pool(name="ps", bufs=4, space="PSUM") as pp:
            w_sb = []
            for kt in range(KT):
                t = wp.tile([128, OS], _f32r, tag=f"w{kt}")
                nc.sync.dma_start(out=t[:], in_=wT[kt * 128:(kt + 1) * 128, :])
                w_sb.append(t)
            b_sb = wp.tile([128, OS], _f32, tag="bias")
            nc.sync.dma_start(out=b_sb[:], in_=bias)

            for mb in range(M // MB):
                xt = xp.tile([128, KT * MB], _f32r, tag="x")
                for kt in range(KT):
                    nc.sync.dma_start(
                        out=xt[:, kt * MB:(kt + 1) * MB],
                        in_=xT[kt * 128:(kt + 1) * 128, mb * MB:(mb + 1) * MB])
                for ms in range(MB // 128):
                    ps = pp.tile([128, OS], _f32, tag="acc")
                    for kt in range(KT):
                        nc.tensor.matmul(
                            ps[:],
                            xt[:, kt * MB + ms * 128:kt * MB + ms * 128 + 128],
                            w_sb[kt][:],
                            start=(kt == 0), stop=(kt == KT - 1))
                    o_sb = op_.tile([128, OS], _f32, tag="o")
                    nc.vector.tensor_add(o_sb[:], ps[:], b_sb[:])
                    row = mb * MB + ms * 128
                    nc.sync.dma_start(out=out[row:row + 128, :], in_=o_sb[:])

    nc.compile()
    return nc


def _compiled():
    global _COMPILED
    if _COMPILED is None:
        _COMPILED = _build()
    return _COMPILED


def _round_fp32r_inplace(a):
    """Round fp32 array to FP32R (round-to-nearest-even to 11 mantissa bits,
    low 12 bits zeroed). Safe for finite data."""
    u = a.view(np.uint32)
    lsb = (u >> 12) & np.uint32(1)
    u += np.uint32(0x7FF)
    u += lsb
    u &= np.uint32(0xFFFFF000)
    return a


def _prep_in_maps(x, W, bias, A, B):
    x = np.asarray(x, dtype=np.float32).reshape(M, K)
    W = np.asarray(W, dtype=np.float32)
    bias = np.asarray(bias, dtype=np.float32)
    A = np.asarray(A, dtype=np.float32)
    B = np.asarray(B, dtype=np.float32)

    weff_t = (W + A @ B).T.copy()            # [K, D] k-major
    _round_fp32r_inplace(weff_t)
    x_t = np.ascontiguousarray(x.T)          # [K, M] k-major
    _round_fp32r_inplace(x_t)

    in_maps = []
    for c in range(N_CORES):
        sl = slice(c * OS, (c + 1) * OS)
        in_maps.append({
            "xT": x_t,
            "wT": np.ascontiguousarray(weff_t[:, sl]),
            "bias": np.tile(bias[sl], (128, 1)),
        })
    return in_maps


def kernel(x, W, bias, A, B):
    nc = _compiled()
    in_maps = _prep_in_maps(x, W, bias, A, B)
    res = run_bass_kernel_spmd(nc, in_maps, core_ids=list(range(N_CORES)),
                               trace=False)
    out = np.concatenate([res.results[c]["out"] for c in range(N_CORES)],
                         axis=1)
    return out.reshape(BATCH, SEQ, D)


# revision 9
# speedup vs baseline: 35.7798x; 35.7798x over previous
"""LoRA linear on 8 Trainium2 NeuronCores.

out = x @ (W + A @ B)^T + bias
  x: [4, 4096, 4096] f32, W: [4096, 4096], bias: [4096], A: [4096, 16], B: [16, 4096]

Strategy (column-parallel / d_out-sharded, per the tensor-parallel pattern):
  - Host: Weff = W + A@B (0.1% of total FLOPs), pre-transpose x and Weff so the
    contraction dim lands on SBUF partitions with no on-chip transposes, round
    both to FP32R (fp32 with 11-bit mantissa; low 12 bits zero) so the PE runs
    matmuls at 1 cycle/row instead of fp32's 4.
  - Each core c: out[:, c*512:(c+1)*512] = x @ WeffT[:, c*512:(c+1)*512] + bias_c.
    WeffT shard (8 MB) stays SBUF-resident; xT streams in m-blocks; 32 k-tile
    matmuls accumulate in PSUM; bias add fused into the PSUM->SBUF evacuation.
"""
import numpy as np

import concourse.bacc as bacc
import concourse.mybir as mybir
import concourse.tile as tile
from concourse.bass_utils import run_bass_kernel_spmd

BATCH, SEQ, D = 4, 4096, 4096
M = BATCH * SEQ          # 16384 rows
K = D                    # contraction
N_CORES = 8
OS = D // N_CORES        # 512 output cols per core
KT = K // 128            # 32 k-tiles
MB = 256                 # m-block rows per x stream tile
XBUFS = 3                # x-block double-buffering depth

_f32 = mybir.dt.float32
_f32r = mybir.dt.float32r

_COMPILED = None


def _build(repeat=1, preload_x=False):
    """repeat>1 wraps the compute in a For_i loop that redundantly recomputes
    the same output -- used only for marginal-cost HW timing (the axon
    dispatch floor is ~80ms, far above the ~1ms kernel)."""
    import contextlib
    nc = bacc.Bacc("TRN2", target_bir_lowering=False, debug=False,
                   num_devices=N_CORES)
    xT = nc.dram_tensor("xT", [K, M], _f32r, kind="ExternalInput").ap()
    wT = nc.dram_tensor("wT", [K, OS], _f32r, kind="ExternalInput").ap()
    bias = nc.dram_tensor("bias", [128, OS], _f32, kind="ExternalInput").ap()
    out = nc.dram_tensor("out", [M, OS], _f32, kind="ExternalOutput").ap()

    with tile.TileContext(nc) as tc:
        with tc.tile_pool(name="w", bufs=1) as wp, \
             tc.tile_pool(name="xb", bufs=XBUFS) as xp, \
             tc.tile_pool(name="ob", bufs=4) as op_, \
             tc.tile_pool(name="ps", bufs=4, space="PSUM") as pp:
            w_sb = []
            for kt in range(KT):
                t = wp.tile([128, OS], _f32r, tag=f"w{kt}")
                nc.sync.dma_start(out=t[:], in_=wT[kt * 128:(kt + 1) * 128, :])
                w_sb.append(t)
            b_sb = wp.tile([128, OS], _f32, tag="bias")
            nc.sync.dma_start(out=b_sb[:], in_=bias)

            pre_xt = None
            if preload_x:
                pre_xt = xp.tile([128, KT * MB], _f32r, tag="x")
                for kt in range(KT):
                    nc.sync.dma_start(
                        out=pre_xt[:, kt * MB:(kt + 1) * MB],
                        in_=xT[kt * 128:(kt + 1) * 128, 0:MB])
            loop_cm = (tc.For_i(0, repeat, 1) if repeat > 1
                       else contextlib.nullcontext())
            with loop_cm:
                _emit_body(nc, tc, xp, op_, pp, xT, out, w_sb, b_sb, pre_xt)

    nc.compile()
    return nc


def _emit_body(nc, tc, xp, op_, pp, xT, out, w_sb, b_sb, pre_xt=None):
    for mb in range(M // MB):
        if pre_xt is not None:
            xt = pre_xt
        else:
            xt = xp.tile([128, KT * MB], _f32r, tag="x")
            # one 3D-AP DMA for the whole [K, MB] block (32 k-tiles):
            # src  [p, kt, j] = xT[kt*128 + p, mb*MB + j]
            # dst  [p, kt, j] = xt[p, kt*MB + j]
            src = xT.rearrange("(kt p) m -> p kt m", p=128)
            nc.sync.dma_start(
                out=xt[:].rearrange("p (kt j) -> p kt j", j=MB),
                in_=src[:, :, mb * MB:(mb + 1) * MB])
        for ms in range(MB // 128):
            ps = pp.tile([128, OS], _f32, tag="acc")
            for kt in range(KT):
                nc.tensor.matmul(
                    ps[:],
                    xt[:, kt * MB + ms * 128:kt * MB + ms * 128 + 128],
                    w_sb[kt][:],
                    start=(kt == 0), stop=(kt == KT - 1))
            o_sb = op_.tile([128, OS], _f32, tag="o")
            nc.vector.tensor_add(o_sb[:], ps[:], b_sb[:])
            row = mb * MB + ms * 128
            nc.sync.dma_start(out=out[row:row + 128, :], in_=o_sb[:])


def _compiled():
    global _COMPILED
    if _COMPILED is None:
        _COMPILED = _build()
    return _COMPILED


def _round_fp32r_inplace(a):
    """Round fp32 array to FP32R (round-to-nearest-even to 11 mantissa bits,
    low 12 bits zeroed). Safe for finite data."""
    u = a.view(np.uint32)
    lsb = (u >> 12) & np.uint32(1)
    u += np.uint32(0x7FF)
    u += lsb
    u &= np.uint32(0xFFFFF000)
    return a


def _prep_in_maps(x, W, bias, A, B):
    x = np.asarray(x, dtype=np.float32).reshape(M, K)
    W = np.asarray(W, dtype=np.float32)
    bias = np.asarray(bias, dtype=np.float32)
    A = np.asarray(A, dtype=np.float32)
    B = np.asarray(B, dtype=np.float32)

    weff_t = (W + A @ B).T.copy()            # [K, D] k-major
    _round_fp32r_inplace(weff_t)
    x_t = np.ascontiguousarray(x.T)          # [K, M] k-major
    _round_fp32r_inplace(x_t)

    in_maps = []
    for c in range(N_CORES):
        sl = slice(c * OS, (c + 1) * OS)
        in_maps.append({
            "xT": x_t,
            "wT": np.ascontiguousarray(weff_t[:, sl]),
            "bias": np.tile(bias[sl], (128, 1)),
        })
    return in_maps


def kernel(x, W, bias, A, B):
    nc = _compiled()
    in_maps = _prep_in_maps(x, W, bias, A, B)
    res = run_bass_kernel_spmd(nc, in_maps, core_ids=list(range(N_CORES)),
                               trace=False)
    out = np.concatenate([res.results[c]["out"] for c in range(N_CORES)],
                         axis=1)
    return out.reshape(BATCH, SEQ, D)
